# revision 16
# baseline (speedup 1.0000x reference)
"""CBOW negative-sampling loss kernel for Trainium2 (8 NeuronCores, SPMD).

Per batch element b: gather 21 rows of 50 floats (10 ctx rows from in_embed,
1 pos + 10 neg from out_embed), context sum, 11 dot products, log-sigmoids,
global mean.

This runtime's indirect DMA consumes ONE offset per partition per op
(HW-verified: multi-offset APs silently use only offset[p, 0] and fetch a
contiguous block), so the kernel issues one indirect_dma_start per
(tile, j): a [128,1] offset column gathers one table row per partition.
21 gathers per 128-element tile, 2688 per core.

Optimizations over the v0 baseline (145.6us):
- Table stored fp8e4m3 with rows padded to 64B (exactly one aligned HBM
  burst per row, half the random-read traffic of fp16) and cast to fp16
  by the DMA during the gather (HW-verified).  Dest rows are 52 elems
  (50 + 2 zero pads from the table padding) so fold halves stay
  4B-aligned for the DVE 2x perf mode.
- The first level of the 10-row context sum happens inside the DMA: ctx
  rows 5..9 are gathered with cce compute_op=add onto the slots holding
  rows 0..4 (issued after all bypass gathers of the group, so the waits
  are pre-satisfied).  Compute batched over groups of 16 tiles: 3 tree
  adds, one broadcast mul (stride-0 AP over the 11 out-rows), folds
  52->26->14 (the 28-stride pad keeps runs 4B-aligned for 2x mode), one
  14->1 reduce - 56 DVE compute instructions/core vs ~2180 baseline.
- pos/neg sign and the /10 context mean are folded into two strided
  sigmoid activations (scale +-0.1); Ln(+eps) with accum_out produces
  the per-partition loss sums in one pass.
Host: loss = -(sum of partials) / B.
"""

import sys

import numpy as np

if "/opt/trn_rl_repo" not in sys.path:
    sys.path.insert(0, "/opt/trn_rl_repo")

from concourse import bass, mybir  # noqa: E402
from concourse import bass_utils  # noqa: E402
from concourse import tile  # noqa: E402
from concourse.bacc import Bacc  # noqa: E402

VOCAB = 50000
DIM = 50
B = 131072
CTX = 10
NEG = 10
NIDX = CTX + 1 + NEG  # 21 rows per batch element: [ctx*10, pos, neg*10]
EPS = 1e-10

NCORES = 8
P = 128
BC = B // NCORES  # 16384
NTILES = BC // P  # 128
TW = 64  # table row width (fp8 bytes, one HBM burst)
DP = 52  # gathered row width in SBUF (50 data + 2 zero pads)
GT = 16  # tiles per compute group

f8 = mybir.dt.float8e4
f16 = mybir.dt.float16
f32 = mybir.dt.float32
i32 = mybir.dt.int32


def build_nc(ntiles: int = NTILES, repeats: int = 1, dump_scores: bool = False):
    nc = Bacc(None, target_bir_lowering=False)
    one_t = nc.alloc_sbuf_tensor("const-one", [P, 1], f32)
    nc.gpsimd.memset(one_t.ap(), 1.0)
    nc.const_aps.aps[(f32, 1.0)] = one_t.ap()
    nc.all_engine_barrier()

    table = nc.dram_tensor("table", [2 * VOCAB, TW], f8, kind="ExternalInput")
    idx = nc.dram_tensor(
        "idx", [P, ntiles * NIDX], mybir.dt.int32, kind="ExternalInput"
    )
    partial = nc.dram_tensor("partial", [P, 1], f32, kind="ExternalOutput")
    scores_out = (
        nc.dram_tensor("scores_out", [P, ntiles * 11], f32, kind="ExternalOutput")
        if dump_scores
        else None
    )

    ngroups = ntiles // GT
    assert ngroups * GT == ntiles

    with tile.TileContext(nc) as tc:
        with (
            tc.tile_pool(name="idxp", bufs=1) as ipool,
            tc.tile_pool(name="gather", bufs=3) as gpool,
            tc.tile_pool(name="work", bufs=1) as wpool,
            tc.tile_pool(name="stage", bufs=1) as spool,
        ):
          for rep in range(repeats):
            it = ipool.tile([P, ntiles * NIDX], i32, tag="it")
            nc.sync.dma_start(out=it[:], in_=idx[:])
            itv = it[:].rearrange("p (t j) -> p t j", t=ntiles, j=NIDX)

            scores = spool.tile([P, ntiles * 11], f32, tag="scores")
            scv = scores[:].rearrange(
                "p (g t j) -> p g t j", g=ngroups, t=GT, j=11
            )

            # shared fold buffer: rows of 28 (26 data + 2 pads); pads are
            # zeroed once so the 28->14 fold stays exact and 4B-aligned
            fb = spool.tile([P, GT * 11 * 28], f16, tag="fb")
            fbv = fb[:].rearrange("p (t j d) -> p t j d", t=GT, j=11, d=28)
            nc.scalar.memzero(fbv[:, :, :, 26:28])

            # slot layout per tile: [ctx0..4, pos, neg0..9] = 16 slots; ctx
            # rows 5..9 are cce=add accumulated onto slots 0..4 by the DMA
            NS = 16
            for g in range(ngroups):
                gt = gpool.tile([P, GT * NS * DP], f16, tag="g")
                gv = gt[:].rearrange(
                    "p (t j d) -> p t j d", t=GT, j=NS, d=DP
                )
                for ti in range(GT):
                    t = g * GT + ti
                    for j in range(NS):
                        nc.gpsimd.indirect_dma_start(
                            out=gv[:, ti, j, :],
                            out_offset=None,
                            in_=table[:],
                            in_offset=bass.IndirectOffsetOnAxis(
                                ap=itv[:, t, j : j + 1], axis=0
                            ),
                        )
                # second ctx row per slot, accumulated in the DMA (the
                # bypass writes above completed hundreds of ops earlier, so
                # these waits never stall the POOL queue)
                for ti in range(GT):
                    t = g * GT + ti
                    for j in range(5):
                        nc.gpsimd.indirect_dma_start(
                            out=gv[:, ti, j, :],
                            out_offset=None,
                            in_=table[:],
                            in_offset=bass.IndirectOffsetOnAxis(
                                ap=itv[:, t, NS + j : NS + j + 1], axis=0
                            ),
                            compute_op=mybir.AluOpType.add,
                        )
                # finish ctx tree-sum: slots 0..4 -> ctx [P, GT, DP]
                s2 = wpool.tile([P, GT * 2 * DP], f16, tag="s2")
                s2v = s2[:].rearrange("p (t k d) -> p t k d", t=GT, k=2, d=DP)
                nc.vector.tensor_add(
                    out=s2v, in0=gv[:, :, 0:2, :], in1=gv[:, :, 2:4, :]
                )
                s3 = wpool.tile([P, GT * DP], f16, tag="s3")
                s3v = s3[:].rearrange("p (t d) -> p t d", t=GT, d=DP)
                nc.vector.tensor_add(
                    out=s3v, in0=s2v[:, :, 0, :], in1=s2v[:, :, 1, :]
                )
                ctx = wpool.tile([P, GT * DP], f16, tag="ctx")
                ctxv = ctx[:].rearrange("p (t d) -> p t d", t=GT, d=DP)
                nc.vector.tensor_add(
                    out=ctxv, in0=s3v, in1=gv[:, :, 4, :]
                )

                # products for slots 5..15 ([pos, neg*10])
                prod = wpool.tile([P, GT * 11 * DP], f16, tag="prod")
                prodv = prod[:].rearrange(
                    "p (t j d) -> p t j d", t=GT, j=11, d=DP
                )
                ctxb = ctxv.unsqueeze(2).broadcast_to((P, GT, 11, DP))
                nc.vector.tensor_mul(
                    out=prodv, in0=gv[:, :, 5:16, :], in1=ctxb
                )
                # fold 52 -> 26 (into 28-stride rows), 28 -> 14, reduce 14 -> 1
                nc.vector.tensor_add(
                    out=fbv[:, :, :, 0:26],
                    in0=prodv[:, :, :, 0:26],
                    in1=prodv[:, :, :, 26:52],
                )
                f2 = wpool.tile([P, GT * 11 * 14], f16, tag="f2")
                f2v = f2[:].rearrange(
                    "p (t j d) -> p t j d", t=GT, j=11, d=14
                )
                nc.vector.tensor_add(
                    out=f2v, in0=fbv[:, :, :, 0:14], in1=fbv[:, :, :, 14:28]
                )
                nc.vector.tensor_reduce(
                    out=scv[:, g, :, :],
                    in_=f2v,
                    axis=mybir.AxisListType.X,
                    op=mybir.AluOpType.add,
                    negate=False,
                )

            acc = spool.tile([P, 1], f32, tag="acc")
            if dump_scores:
                nc.sync.dma_start(out=scores_out[:], in_=scores[:])
            sall = scores[:].rearrange("p (t j) -> p t j", t=ntiles, j=11)
            # -log sig(pos_s) = softplus(-pos_s), -log sig(-neg_s) =
            # softplus(neg_s); softplus(x) = Ln(1 + Exp(x)) keeps both
            # activations in the natural_log_exp table set (one load).
            nc.scalar.activation(
                out=sall[:, :, 0:1],
                in_=sall[:, :, 0:1],
                func=mybir.ActivationFunctionType.Exp,
                scale=-0.1,
            )
            nc.scalar.activation(
                out=sall[:, :, 1:11],
                in_=sall[:, :, 1:11],
                func=mybir.ActivationFunctionType.Exp,
                scale=0.1,
            )
            nc.scalar.activation(
                out=scores[:],
                in_=scores[:],
                func=mybir.ActivationFunctionType.Ln,
                bias=1.0,
                accum_out=acc[:],
            )
            nc.sync.dma_start(out=partial[:], in_=acc[:])

    nc.compile()
    return nc


def _prep_inputs(context_idxs, pos_target, neg_samples, in_embed_W, out_embed_W):
    ci = np.asarray(context_idxs, dtype=np.int64)
    idx_all = np.concatenate(
        [
            ci[:, 0:5],
            np.asarray(pos_target, dtype=np.int64)[:, None] + VOCAB,
            np.asarray(neg_samples, dtype=np.int64) + VOCAB,
            ci[:, 5:10],
        ],
        axis=1,
    ).astype(np.int32)  # [B, 21] = [ctx0..4, pos, neg*10, ctx5..9]

    table = np.zeros((2 * VOCAB, TW), dtype=mybir.dt.np(f8))
    table[:VOCAB, :DIM] = np.asarray(in_embed_W).astype(mybir.dt.np(f8))
    table[VOCAB:, :DIM] = np.asarray(out_embed_W).astype(mybir.dt.np(f8))

    in_maps = []
    for c in range(NCORES):
        sl = idx_all[c * BC : (c + 1) * BC]
        idx_c = (
            sl.reshape(NTILES, P, NIDX)
            .transpose(1, 0, 2)
            .reshape(P, NTILES * NIDX)
            .copy()
        )
        in_maps.append({"table": table, "idx": idx_c})
    return in_maps


def kernel(context_idxs, pos_target, neg_samples, in_embed_W, out_embed_W):
    in_maps = _prep_inputs(
        context_idxs, pos_target, neg_samples, in_embed_W, out_embed_W
    )
    nc = build_nc()
    res = bass_utils.run_bass_kernel_spmd(nc, in_maps, core_ids=list(range(NCORES)))
    # partials are sums of softplus terms = -(log-sigmoid sums), so the
    # loss is +total/B
    total = sum(float(r["partial"].sum()) for r in res.results)
    return np.float32(total / B)


# revision 18
# speedup vs baseline: 1.2005x; 1.2005x over previous
"""CBOW negative-sampling loss kernel for Trainium2 (8 NeuronCores, SPMD).

Per batch element b: gather 21 rows of 50 floats (10 ctx rows from in_embed,
1 pos + 10 neg from out_embed), context sum, 11 dot products, log-sigmoids,
global mean.

This runtime's indirect DMA consumes ONE offset per partition per op
(HW-verified: multi-offset APs silently use only offset[p, 0] and fetch a
contiguous block), so the kernel issues one indirect_dma_start per
(tile, j): a [128,1] offset column gathers one table row per partition.
21 gathers per 128-element tile, 2688 per core.

Optimizations over the v0 baseline (145.6us):
- Table stored fp8e4m3 with rows padded to 64B (exactly one aligned HBM
  burst per row, half the random-read traffic of fp16) and cast to fp16
  by the DMA during the gather (HW-verified).  Dest rows are 52 elems
  (50 + 2 zero pads from the table padding) so fold halves stay
  4B-aligned for the DVE 2x perf mode.
- The first level of the 10-row context sum happens inside the DMA: ctx
  rows 5..9 are gathered with cce compute_op=add onto the slots holding
  rows 0..4 (issued after all bypass gathers of the group, so the waits
  are pre-satisfied).  Compute batched over groups of 16 tiles: 3 tree
  adds, one broadcast mul (stride-0 AP over the 11 out-rows), folds
  52->26->14 (the 28-stride pad keeps runs 4B-aligned for 2x mode), one
  14->1 reduce - 56 DVE compute instructions/core vs ~2180 baseline.
- Tail: -log sig(+-s) == softplus(-+s) here (scores never reach the
  1e-10 eps regime), computed as Ln(1 + Exp(-+0.1*s)) so the pos/neg
  sign and the /10 context mean fold into the Exp scale and both
  activations share ONE table set (natural_log_exp); Ln's accum_out
  yields the per-partition loss sums.
Host: loss = +(sum of partials) / B  (softplus sums are positive).
"""

import sys

import numpy as np

if "/opt/trn_rl_repo" not in sys.path:
    sys.path.insert(0, "/opt/trn_rl_repo")

from concourse import bass, mybir  # noqa: E402
from concourse import bass_utils  # noqa: E402
from concourse import tile  # noqa: E402
from concourse.bacc import Bacc  # noqa: E402

VOCAB = 50000
DIM = 50
B = 131072
CTX = 10
NEG = 10
NIDX = CTX + 1 + NEG  # 21 rows per batch element

NCORES = 8
P = 128
BC = B // NCORES  # 16384
NTILES = BC // P  # 128
TW = 64  # table row width (fp8 bytes, one HBM burst)
DP = 52  # gathered row width in SBUF (50 data + 2 zero pads)
GT = 16  # tiles per compute group

f8 = mybir.dt.float8e4
f16 = mybir.dt.float16
f32 = mybir.dt.float32
i32 = mybir.dt.int32


def build_nc(ntiles: int = NTILES, repeats: int = 1, dump_scores: bool = False):
    nc = Bacc(None, target_bir_lowering=False)
    one_t = nc.alloc_sbuf_tensor("const-one", [P, 1], f32)
    nc.gpsimd.memset(one_t.ap(), 1.0)
    nc.const_aps.aps[(f32, 1.0)] = one_t.ap()
    nc.all_engine_barrier()

    table = nc.dram_tensor("table", [2 * VOCAB, TW], f8, kind="ExternalInput")
    idx = nc.dram_tensor(
        "idx", [P, ntiles * NIDX], mybir.dt.int32, kind="ExternalInput"
    )
    partial = nc.dram_tensor("partial", [P, 1], f32, kind="ExternalOutput")
    scores_out = (
        nc.dram_tensor("scores_out", [P, ntiles * 11], f32, kind="ExternalOutput")
        if dump_scores
        else None
    )

    ngroups = ntiles // GT
    assert ngroups * GT == ntiles

    with tile.TileContext(nc) as tc:
        with (
            tc.tile_pool(name="idxp", bufs=1) as ipool,
            tc.tile_pool(name="gather", bufs=3) as gpool,
            tc.tile_pool(name="work", bufs=1) as wpool,
            tc.tile_pool(name="stage", bufs=1) as spool,
        ):
          for rep in range(repeats):
            it = ipool.tile([P, ntiles * NIDX], i32, tag="it")
            nc.sync.dma_start(out=it[:], in_=idx[:])
            itv = it[:].rearrange("p (t j) -> p t j", t=ntiles, j=NIDX)

            scores = spool.tile([P, ntiles * 11], f32, tag="scores")
            scv = scores[:].rearrange(
                "p (g t j) -> p g t j", g=ngroups, t=GT, j=11
            )

            # shared fold buffer: rows of 28 (26 data + 2 pads); pads are
            # zeroed once so the 28->14 fold stays exact and 4B-aligned
            fb = spool.tile([P, GT * 11 * 28], f16, tag="fb")
            fbv = fb[:].rearrange("p (t j d) -> p t j d", t=GT, j=11, d=28)
            nc.scalar.memzero(fbv[:, :, :, 26:28])

            # slot layout per tile: [ctx0..4, pos, neg0..9] = 16 slots; ctx
            # rows 5..9 are cce=add accumulated onto slots 0..4 by the DMA
            NS = 16
            for g in range(ngroups):
                gt = gpool.tile([P, GT * NS * DP], f16, tag="g")
                gv = gt[:].rearrange(
                    "p (t j d) -> p t j d", t=GT, j=NS, d=DP
                )
                for ti in range(GT):
                    t = g * GT + ti
                    for j in range(NS):
                        nc.gpsimd.indirect_dma_start(
                            out=gv[:, ti, j, :],
                            out_offset=None,
                            in_=table[:],
                            in_offset=bass.IndirectOffsetOnAxis(
                                ap=itv[:, t, j : j + 1], axis=0
                            ),
                        )
                # second ctx row per slot, accumulated in the DMA (the
                # bypass writes above completed hundreds of ops earlier, so
                # these waits never stall the POOL queue)
                for ti in range(GT):
                    t = g * GT + ti
                    for j in range(5):
                        nc.gpsimd.indirect_dma_start(
                            out=gv[:, ti, j, :],
                            out_offset=None,
                            in_=table[:],
                            in_offset=bass.IndirectOffsetOnAxis(
                                ap=itv[:, t, NS + j : NS + j + 1], axis=0
                            ),
                            compute_op=mybir.AluOpType.add,
                        )
                # finish ctx tree-sum: slots 0..4 -> ctx [P, GT, DP]
                s2 = wpool.tile([P, GT * 2 * DP], f16, tag="s2")
                s2v = s2[:].rearrange("p (t k d) -> p t k d", t=GT, k=2, d=DP)
                nc.vector.tensor_add(
                    out=s2v, in0=gv[:, :, 0:2, :], in1=gv[:, :, 2:4, :]
                )
                s3 = wpool.tile([P, GT * DP], f16, tag="s3")
                s3v = s3[:].rearrange("p (t d) -> p t d", t=GT, d=DP)
                nc.vector.tensor_add(
                    out=s3v, in0=s2v[:, :, 0, :], in1=s2v[:, :, 1, :]
                )
                ctx = wpool.tile([P, GT * DP], f16, tag="ctx")
                ctxv = ctx[:].rearrange("p (t d) -> p t d", t=GT, d=DP)
                nc.vector.tensor_add(
                    out=ctxv, in0=s3v, in1=gv[:, :, 4, :]
                )

                # products for slots 5..15 ([pos, neg*10])
                prod = wpool.tile([P, GT * 11 * DP], f16, tag="prod")
                prodv = prod[:].rearrange(
                    "p (t j d) -> p t j d", t=GT, j=11, d=DP
                )
                ctxb = ctxv.unsqueeze(2).broadcast_to((P, GT, 11, DP))
                nc.vector.tensor_mul(
                    out=prodv, in0=gv[:, :, 5:16, :], in1=ctxb
                )
                # fold 52 -> 26 (into 28-stride rows), 28 -> 14, reduce 14 -> 1
                nc.vector.tensor_add(
                    out=fbv[:, :, :, 0:26],
                    in0=prodv[:, :, :, 0:26],
                    in1=prodv[:, :, :, 26:52],
                )
                f2 = wpool.tile([P, GT * 11 * 14], f16, tag="f2")
                f2v = f2[:].rearrange(
                    "p (t j d) -> p t j d", t=GT, j=11, d=14
                )
                nc.vector.tensor_add(
                    out=f2v, in0=fbv[:, :, :, 0:14], in1=fbv[:, :, :, 14:28]
                )
                nc.vector.tensor_reduce(
                    out=scv[:, g, :, :],
                    in_=f2v,
                    axis=mybir.AxisListType.X,
                    op=mybir.AluOpType.add,
                    negate=False,
                )

            acc = spool.tile([P, 1], f32, tag="acc")
            if dump_scores:
                nc.sync.dma_start(out=scores_out[:], in_=scores[:])
            sall = scores[:].rearrange("p (t j) -> p t j", t=ntiles, j=11)
            # -log sig(pos_s) = softplus(-pos_s), -log sig(-neg_s) =
            # softplus(neg_s); softplus(x) = Ln(1 + Exp(x)) keeps both
            # activations in the natural_log_exp table set (one load).
            nc.scalar.activation(
                out=sall[:, :, 0:1],
                in_=sall[:, :, 0:1],
                func=mybir.ActivationFunctionType.Exp,
                scale=-0.1,
            )
            nc.scalar.activation(
                out=sall[:, :, 1:11],
                in_=sall[:, :, 1:11],
                func=mybir.ActivationFunctionType.Exp,
                scale=0.1,
            )
            nc.scalar.activation(
                out=scores[:],
                in_=scores[:],
                func=mybir.ActivationFunctionType.Ln,
                bias=1.0,
                accum_out=acc[:],
            )
            nc.sync.dma_start(out=partial[:], in_=acc[:])

    nc.compile()
    return nc


def _prep_inputs(context_idxs, pos_target, neg_samples, in_embed_W, out_embed_W):
    ci = np.asarray(context_idxs, dtype=np.int64)
    idx_all = np.concatenate(
        [
            ci[:, 0:5],
            np.asarray(pos_target, dtype=np.int64)[:, None] + VOCAB,
            np.asarray(neg_samples, dtype=np.int64) + VOCAB,
            ci[:, 5:10],
        ],
        axis=1,
    ).astype(np.int32)  # [B, 21] = [ctx0..4, pos, neg*10, ctx5..9]

    table = np.zeros((2 * VOCAB, TW), dtype=mybir.dt.np(f8))
    table[:VOCAB, :DIM] = np.asarray(in_embed_W).astype(mybir.dt.np(f8))
    table[VOCAB:, :DIM] = np.asarray(out_embed_W).astype(mybir.dt.np(f8))

    in_maps = []
    for c in range(NCORES):
        sl = idx_all[c * BC : (c + 1) * BC]
        idx_c = (
            sl.reshape(NTILES, P, NIDX)
            .transpose(1, 0, 2)
            .reshape(P, NTILES * NIDX)
            .copy()
        )
        in_maps.append({"table": table, "idx": idx_c})
    return in_maps


def kernel(context_idxs, pos_target, neg_samples, in_embed_W, out_embed_W):
    in_maps = _prep_inputs(
        context_idxs, pos_target, neg_samples, in_embed_W, out_embed_W
    )
    nc = build_nc()
    res = bass_utils.run_bass_kernel_spmd(nc, in_maps, core_ids=list(range(NCORES)))
    # partials are sums of softplus terms = -(log-sigmoid sums), so the
    # loss is +total/B
    total = sum(float(r["partial"].sum()) for r in res.results)
    return np.float32(total / B)
